# revision 13
# baseline (speedup 1.0000x reference)
"""Penalty-weighted Huber loss on 8 TRN2 NeuronCores (data parallel).

result = mean(huber(y_pred - y_true) * LUT[y_true]),  N = 16,777,216
  huber(d) = 0.5*d^2            if |d| < 0.5
           = 0.5*(|d| - 0.25)   else
  LUT = [1, 5, 4, 2]

Identities used on device:
  huber2(d) = 2*huber(d) = m*(2a - m),  a = |d|, m = min(a, 0.5)
  w(t) = ((2/3*t - 4.5)*t + 47/6)*t + 1  (exact at t in {0,1,2,3})
  out = w(t)*h2 = poly3(t)*h2 + h2   -> sum via DVE accum; host / (2N)

Each core handles a contiguous 2^21-element slice viewed as [128, 16384].
Two custom DVE ops do the whole elementwise pipeline in 2 passes; ACT
casts int32->f32; partial sums [128, n_tiles] DMA back; host reduces.
"""

from operator import add

import numpy as np

from concourse import bacc, bass, tile
from concourse import dve_ops
from concourse.bass import mybir
from concourse.bass_utils import run_bass_kernel_spmd
from concourse.dve_spec import (
    C0,
    C1,
    C2,
    Spec,
    Src0,
    Src1,
    Zero,
    _has_src1,
    lower,
    maxx,
    minn,
)
from concourse.dve_uop import DveOpSpec

N = 16777216
NCORES = 8
PER_CORE = N // NCORES          # 2097152
P = 128
W = PER_CORE // P               # 16384
F = 4096                        # tile free dim
NT = W // F                     # 4 tiles per core

DELTA = 0.5
# w(t) = ((PC0*t + PC1)*t + PC2)*t + 1 hits [1,5,4,2] at t=0..3
PC0 = float(np.float32(2.0 / 3.0))
PC1 = -4.5
PC2 = float(np.float32(47.0 / 6.0))


def _register(name: str, spec: Spec, subdim: bool = False) -> dve_ops.DveOp:
    if name in dve_ops._SUB_OPCODE_FOR_NAME:
        return next(op for op in dve_ops.OPS if op.name == name)
    shas = {}
    for ver in ("v3", "v4"):
        tmp = DveOpSpec(
            name=name, opcode=1, uops=lower(spec, ver=ver), rd1_en=_has_src1(spec)
        )
        shas[ver] = tmp.sha(ver)
    op = dve_ops.DveOp(name, spec, subdim, shas)
    dve_ops.OPS.append(op)
    dve_ops.CUSTOM_DVE_SPECS[name] = spec
    dve_ops._SUB_OPCODE_FOR_NAME[name] = (
        dve_ops._CUSTOM_DVE_ROW_BASE + len(dve_ops.OPS) - 1
    )
    return op


def _ref_huber2(in0, in1, s0, s1, imm2):
    d = in0.astype(np.float32) - in1.astype(np.float32)
    a = np.abs(d)
    m = np.minimum(a, np.float32(s0))
    return (m * (2.0 * a - m)).astype(np.float32)


_d = Src0 - Src1
_a = maxx(_d, -_d)
_m = minn(_a, C0)
HUBER2_PH = _register(
    "HUBER2_PH_ANT",
    Spec(body=_m * ((_a + _a) - _m), reference=_ref_huber2),
)


def _ref_wpoly_mr(in0, in1, s0, s1, imm2):
    t = in1.astype(np.float32)
    w = ((np.float32(s0) * t + np.float32(s1)) * t + np.float32(imm2)) * t + 1.0
    b = (w * in0.astype(np.float32)).astype(np.float32)
    return b, b.reshape(b.shape[0], -1).sum(axis=-1, keepdims=True)


_u = ((C0 * Src1 + C1) * Src1 + C2) * Src1
WPOLY_MR = _register(
    "WPOLY_MR_PH_ANT",
    Spec(body=_u * Src0 + Src0, accum=add, accum_init=Zero, reference=_ref_wpoly_mr),
)


def build_program(repeat: int = 1) -> bass.Bass:
    # Bacc owns the raw-Bass-missing passes: generate_event_semaphores
    # (TRN2 allows at most 1 embedded wait per instruction) and
    # codegen_inst_isa_subclasses (populates InstCustomDveAnt .instr bytes).
    # y_true arrives pre-cast to f32 by the host (same byte count as int32),
    # so ACT issues only DMA enqueues and never blocks its ring.
    nc = bacc.Bacc("TRN2", target_bir_lowering=False, debug=False)
    yp = nc.declare_dram_parameter("y_pred", [P, W], mybir.dt.float32, isOutput=False)
    yt = nc.declare_dram_parameter("y_true", [P, W], mybir.dt.float32, isOutput=False)
    po = nc.declare_dram_parameter("partials", [P, NT], mybir.dt.float32, isOutput=True)

    with tile.TileContext(nc) as tc:
        with (
            tc.tile_pool(name="pred", bufs=3) as pred_pool,
            tc.tile_pool(name="tf", bufs=3) as tf_pool,
            tc.tile_pool(name="h2", bufs=1) as h2_pool,
            tc.tile_pool(name="scratch", bufs=1) as scratch_pool,
            tc.tile_pool(name="acc", bufs=1) as acc_pool,
        ):
            partials = acc_pool.tile([P, NT], mybir.dt.float32)
            for i in range(NT * repeat):
                i = i % NT
                pt = pred_pool.tile([P, F], mybir.dt.float32)
                nc.sync.dma_start(pt[:], yp[:, bass.ts(i, F)])
                ft = tf_pool.tile([P, F], mybir.dt.float32)
                nc.scalar.dma_start(ft[:], yt[:, bass.ts(i, F)])

                h2 = h2_pool.tile([P, F], mybir.dt.float32)
                nc.vector._custom_dve(
                    HUBER2_PH, out=h2[:], in0=pt[:], in1=ft[:], s0=DELTA
                )
                sc = scratch_pool.tile([P, F], mybir.dt.float32)
                nc.vector._custom_dve(
                    WPOLY_MR,
                    out=sc[:],
                    in0=h2[:],
                    in1=ft[:],
                    s0=PC0,
                    s1=PC1,
                    imm2=PC2,
                    accum_out=partials[:, i : i + 1],
                )
            nc.sync.dma_start(po[:], partials[:])
    nc.compile()
    return nc


def kernel(y_pred: np.ndarray, y_true: np.ndarray) -> np.ndarray:
    y_pred = np.ascontiguousarray(np.asarray(y_pred, dtype=np.float32)).reshape(
        NCORES, P, W
    )
    y_true = np.ascontiguousarray(np.asarray(y_true).astype(np.float32)).reshape(
        NCORES, P, W
    )

    nc = build_program()
    in_maps = [
        {"y_pred": y_pred[c], "y_true": y_true[c]} for c in range(NCORES)
    ]
    res = run_bass_kernel_spmd(nc, in_maps, list(range(NCORES)))
    total = 0.0
    for c in range(NCORES):
        total += res.results[c]["partials"].astype(np.float64).sum()
    return np.asarray(total / (2.0 * N), dtype=np.float32)


# revision 14
# speedup vs baseline: 1.2540x; 1.2540x over previous
"""Penalty-weighted Huber loss on 8 TRN2 NeuronCores (data parallel).

result = mean(huber(y_pred - y_true) * LUT[y_true]),  N = 16,777,216
  huber(d) = 0.5*d^2            if |d| < 0.5
           = 0.5*(|d| - 0.25)   else
  LUT = [1, 5, 4, 2]

Identities used on device:
  huber2(d) = 2*huber(d) = m*(2a - m),  a = |d|, m = min(a, 0.5)
  w(t) = ((2/3*t - 4.5)*t + 47/6)*t + 1  (exact at t in {0,1,2,3})
  out = w(t)*h2 = poly3(t)*h2 + h2   -> sum via DVE accum; host / (2N)

Each core handles a contiguous 2^21-element slice viewed as [128, 16384].
Two custom DVE ops do the whole elementwise pipeline in 2 passes; ACT
casts int32->f32; partial sums [128, n_tiles] DMA back; host reduces.
"""

from operator import add

import numpy as np

from concourse import bacc, bass, tile
from concourse import dve_ops
from concourse.bass import mybir
from concourse.bass_utils import run_bass_kernel_spmd
from concourse.dve_spec import (
    C0,
    C1,
    C2,
    Spec,
    Src0,
    Src1,
    Zero,
    _has_src1,
    lower,
    maxx,
    minn,
)
from concourse.dve_uop import DveOpSpec

N = 16777216
NCORES = 8
PER_CORE = N // NCORES          # 2097152
P = 128
W = PER_CORE // P               # 16384
F = 4096                        # tile free dim
NT = W // F                     # 4 tiles per core

DELTA = 0.5
# w(t) = ((PC0*t + PC1)*t + PC2)*t + 1 hits [1,5,4,2] at t=0..3
PC0 = float(np.float32(2.0 / 3.0))
PC1 = -4.5
PC2 = float(np.float32(47.0 / 6.0))


def _register(name: str, spec: Spec, subdim: bool = False) -> dve_ops.DveOp:
    if name in dve_ops._SUB_OPCODE_FOR_NAME:
        return next(op for op in dve_ops.OPS if op.name == name)
    shas = {}
    for ver in ("v3", "v4"):
        tmp = DveOpSpec(
            name=name, opcode=1, uops=lower(spec, ver=ver), rd1_en=_has_src1(spec)
        )
        shas[ver] = tmp.sha(ver)
    op = dve_ops.DveOp(name, spec, subdim, shas)
    dve_ops.OPS.append(op)
    dve_ops.CUSTOM_DVE_SPECS[name] = spec
    dve_ops._SUB_OPCODE_FOR_NAME[name] = (
        dve_ops._CUSTOM_DVE_ROW_BASE + len(dve_ops.OPS) - 1
    )
    return op


def _ref_huber2(in0, in1, s0, s1, imm2):
    d = in0.astype(np.float32) - in1.astype(np.float32)
    a = np.abs(d)
    m = np.minimum(a, np.float32(s0))
    return (m * (2.0 * a - m)).astype(np.float32)


_d = Src0 - Src1
_a = maxx(_d, -_d)
_m = minn(_a, C0)
HUBER2_PH = _register(
    "HUBER2_PH_ANT",
    Spec(body=_m * ((_a + _a) - _m), reference=_ref_huber2),
)


def _ref_wpoly_mr(in0, in1, s0, s1, imm2):
    t = in1.astype(np.float32)
    w = ((np.float32(s0) * t + np.float32(s1)) * t + np.float32(imm2)) * t + 1.0
    b = (w * in0.astype(np.float32)).astype(np.float32)
    return b, b.reshape(b.shape[0], -1).sum(axis=-1, keepdims=True)


_u = ((C0 * Src1 + C1) * Src1 + C2) * Src1
WPOLY_MR = _register(
    "WPOLY_MR_PH_ANT",
    Spec(body=_u * Src0 + Src0, accum=add, accum_init=Zero, reference=_ref_wpoly_mr),
)


def build_program(repeat: int = 1) -> bass.Bass:
    # Bacc owns the raw-Bass-missing passes: generate_event_semaphores
    # (TRN2 allows at most 1 embedded wait per instruction) and
    # codegen_inst_isa_subclasses (populates InstCustomDveAnt .instr bytes).
    # y_true arrives pre-cast to f32 by the host (same byte count as int32),
    # so ACT issues only DMA enqueues and never blocks its ring.
    nc = bacc.Bacc("TRN2", target_bir_lowering=False, debug=False)
    yp = nc.declare_dram_parameter("y_pred", [P, W], mybir.dt.float32, isOutput=False)
    yt = nc.declare_dram_parameter("y_true", [P, W], mybir.dt.float32, isOutput=False)
    po = nc.declare_dram_parameter("partials", [P, NT], mybir.dt.float32, isOutput=True)

    with tile.TileContext(nc) as tc:
        with (
            tc.tile_pool(name="pred", bufs=3) as pred_pool,
            tc.tile_pool(name="tf", bufs=3) as tf_pool,
            tc.tile_pool(name="h2", bufs=2) as h2_pool,
            tc.tile_pool(name="scratch", bufs=2) as scratch_pool,
            tc.tile_pool(name="acc", bufs=1) as acc_pool,
        ):
            partials = acc_pool.tile([P, NT], mybir.dt.float32)
            for i in range(NT * repeat):
                i = i % NT
                pt = pred_pool.tile([P, F], mybir.dt.float32)
                nc.sync.dma_start(pt[:], yp[:, bass.ts(i, F)])
                ft = tf_pool.tile([P, F], mybir.dt.float32)
                nc.scalar.dma_start(ft[:], yt[:, bass.ts(i, F)])

                h2 = h2_pool.tile([P, F], mybir.dt.float32)
                nc.vector._custom_dve(
                    HUBER2_PH, out=h2[:], in0=pt[:], in1=ft[:], s0=DELTA
                )
                sc = scratch_pool.tile([P, F], mybir.dt.float32)
                nc.vector._custom_dve(
                    WPOLY_MR,
                    out=sc[:],
                    in0=h2[:],
                    in1=ft[:],
                    s0=PC0,
                    s1=PC1,
                    imm2=PC2,
                    accum_out=partials[:, i : i + 1],
                )
            nc.sync.dma_start(po[:], partials[:])
    nc.compile()
    return nc


def kernel(y_pred: np.ndarray, y_true: np.ndarray) -> np.ndarray:
    y_pred = np.ascontiguousarray(np.asarray(y_pred, dtype=np.float32)).reshape(
        NCORES, P, W
    )
    y_true = np.ascontiguousarray(np.asarray(y_true).astype(np.float32)).reshape(
        NCORES, P, W
    )

    nc = build_program()
    in_maps = [
        {"y_pred": y_pred[c], "y_true": y_true[c]} for c in range(NCORES)
    ]
    res = run_bass_kernel_spmd(nc, in_maps, list(range(NCORES)))
    total = 0.0
    for c in range(NCORES):
        total += res.results[c]["partials"].astype(np.float64).sum()
    return np.asarray(total / (2.0 * N), dtype=np.float32)


# revision 18
# speedup vs baseline: 1.3396x; 1.0682x over previous
"""Penalty-weighted Huber loss on 8 TRN2 NeuronCores (data parallel).

result = mean(huber(y_pred - y_true) * LUT[y_true]),  N = 16,777,216
  huber(d) = 0.5*d^2            if |d| < 0.5
           = 0.5*(|d| - 0.25)   else
  LUT = [1, 5, 4, 2]

Identities used on device:
  huber2(d) = 2*huber(d) = m*(2a - m),  a = |d|, m = min(a, 0.5)
  w(t) = ((2/3*t - 4.5)*t + 47/6)*t + 1  (exact at t in {0,1,2,3})
  out = w(t)*h2 = poly3(t)*h2 + h2   -> sum via DVE accum; host / (2N)

Each core handles a contiguous 2^21-element slice viewed as [128, 16384].
Two custom DVE ops do the whole elementwise pipeline in 2 passes; ACT
casts int32->f32; partial sums [128, n_tiles] DMA back; host reduces.
"""

from operator import add

import ml_dtypes
import numpy as np

from concourse import bacc, bass, tile
from concourse import dve_ops
from concourse.bass import mybir
from concourse.bass_utils import run_bass_kernel_spmd
from concourse.dve_spec import (
    C0,
    C1,
    C2,
    Spec,
    Src0,
    Src1,
    Zero,
    _has_src1,
    lower,
    maxx,
    minn,
)
from concourse.dve_uop import DveOpSpec

N = 16777216
NCORES = 8
PER_CORE = N // NCORES          # 2097152
P = 128
W = PER_CORE // P               # 16384
F = 8192                        # tile free dim
NT = W // F                     # 2 tiles per core

DELTA = 0.5
# w(t) = ((PC0*t + PC1)*t + PC2)*t + 1 hits [1,5,4,2] at t=0..3
PC0 = float(np.float32(2.0 / 3.0))
PC1 = -4.5
PC2 = float(np.float32(47.0 / 6.0))


def _register(name: str, spec: Spec, subdim: bool = False) -> dve_ops.DveOp:
    if name in dve_ops._SUB_OPCODE_FOR_NAME:
        return next(op for op in dve_ops.OPS if op.name == name)
    shas = {}
    for ver in ("v3", "v4"):
        tmp = DveOpSpec(
            name=name, opcode=1, uops=lower(spec, ver=ver), rd1_en=_has_src1(spec)
        )
        shas[ver] = tmp.sha(ver)
    op = dve_ops.DveOp(name, spec, subdim, shas)
    dve_ops.OPS.append(op)
    dve_ops.CUSTOM_DVE_SPECS[name] = spec
    dve_ops._SUB_OPCODE_FOR_NAME[name] = (
        dve_ops._CUSTOM_DVE_ROW_BASE + len(dve_ops.OPS) - 1
    )
    return op


def _ref_huber2(in0, in1, s0, s1, imm2):
    d = in0.astype(np.float32) - in1.astype(np.float32)
    a = np.abs(d)
    m = np.minimum(a, np.float32(s0))
    return (m * (2.0 * a - m)).astype(np.float32)


_d = Src0 - Src1
_a = maxx(_d, -_d)
_m = minn(_a, C0)
HUBER2_PH = _register(
    "HUBER2_PH_ANT",
    Spec(body=_m * ((_a + _a) - _m), reference=_ref_huber2),
)


def _ref_wpoly_mr(in0, in1, s0, s1, imm2):
    t = in1.astype(np.float32)
    w = ((np.float32(s0) * t + np.float32(s1)) * t + np.float32(imm2)) * t + 1.0
    b = (w * in0.astype(np.float32)).astype(np.float32)
    return b, b.reshape(b.shape[0], -1).sum(axis=-1, keepdims=True)


_u = ((C0 * Src1 + C1) * Src1 + C2) * Src1
WPOLY_MR = _register(
    "WPOLY_MR_PH_ANT",
    Spec(body=_u * Src0 + Src0, accum=add, accum_init=Zero, reference=_ref_wpoly_mr),
)


def build_program(repeat: int = 1) -> bass.Bass:
    # Bacc owns the raw-Bass-missing passes: generate_event_semaphores
    # (TRN2 allows at most 1 embedded wait per instruction) and
    # codegen_inst_isa_subclasses (populates InstCustomDveAnt .instr bytes).
    # y_true arrives pre-cast to f32 by the host (same byte count as int32),
    # so ACT issues only DMA enqueues and never blocks its ring.
    nc = bacc.Bacc("TRN2", target_bir_lowering=False, debug=False)
    yp = nc.declare_dram_parameter("y_pred", [P, W], mybir.dt.float32, isOutput=False)
    yt = nc.declare_dram_parameter("y_true", [P, W], mybir.dt.bfloat16, isOutput=False)
    po = nc.declare_dram_parameter("partials", [P, NT], mybir.dt.float32, isOutput=True)

    with tile.TileContext(nc) as tc:
        with (
            tc.tile_pool(name="pred", bufs=2) as pred_pool,
            tc.tile_pool(name="tf", bufs=2) as tf_pool,
            tc.tile_pool(name="h2", bufs=2) as h2_pool,
            tc.tile_pool(name="acc", bufs=1) as acc_pool,
        ):
            partials = acc_pool.tile([P, NT], mybir.dt.float32)
            for i in range(NT * repeat):
                i = i % NT
                pt = pred_pool.tile([P, F], mybir.dt.float32)
                nc.sync.dma_start(pt[:], yp[:, bass.ts(i, F)])
                ft = tf_pool.tile([P, F], mybir.dt.bfloat16)
                nc.scalar.dma_start(ft[:], yt[:, bass.ts(i, F)])

                h2 = h2_pool.tile([P, F], mybir.dt.float32)
                nc.vector._custom_dve(
                    HUBER2_PH, out=h2[:], in0=pt[:], in1=ft[:], s0=DELTA
                )
                # pass B writes its (unused) body output in place over h2;
                # per-element write lags the read by the pipeline depth.
                nc.vector._custom_dve(
                    WPOLY_MR,
                    out=h2[:],
                    in0=h2[:],
                    in1=ft[:],
                    s0=PC0,
                    s1=PC1,
                    imm2=PC2,
                    accum_out=partials[:, i : i + 1],
                )
            nc.sync.dma_start(po[:], partials[:])
    nc.compile()
    return nc


def kernel(y_pred: np.ndarray, y_true: np.ndarray) -> np.ndarray:
    y_pred = np.ascontiguousarray(np.asarray(y_pred, dtype=np.float32)).reshape(
        NCORES, P, W
    )
    # y_true values are {0,1,2,3}: exact in bf16, halves its DMA traffic
    y_true = np.ascontiguousarray(
        np.asarray(y_true).astype(ml_dtypes.bfloat16)
    ).reshape(NCORES, P, W)

    nc = build_program()
    in_maps = [
        {"y_pred": y_pred[c], "y_true": y_true[c]} for c in range(NCORES)
    ]
    res = run_bass_kernel_spmd(nc, in_maps, list(range(NCORES)))
    total = 0.0
    for c in range(NCORES):
        total += res.results[c]["partials"].astype(np.float64).sum()
    return np.asarray(total / (2.0 * N), dtype=np.float32)
